# revision 27
# baseline (speedup 1.0000x reference)
# SDPA (naive, additive mask) for TRN2, 8 NeuronCores.
#
# Full problem: q/k/v [16, 4096, 64] f32, mask [4096, 4096] f32
#   out = softmax(q @ k^T / 8 + mask) @ v
#
# Sharding (2 head-groups x 4 q-groups = 8 cores, minimizes HBM traffic):
#   core c: hg, qg = divmod(c, 4)
#   heads hg*8:(hg+1)*8, q-rows qg*1024:(qg+1)*1024, k/v full, mask q-slice.
#
# Kernel (per core, flash-style with transposed scores):
#   E^T = exp(mask^T) resident in SBUF (bf16)  [t-major so softmax sum is a
#   matmul axis]; per head: scores^T = K^T.T @ Q^T on PE (bf16), exp on ACT
#   (scale=0.125 folded in), multiply by E^T on DVE (all-bf16 packed mode),
#   then PV = [V | ones].T @ attn^T accumulated in PSUM -> unnormalized out^T
#   plus softmax denominators in the last row; transpose back on PE, scale by
#   reciprocal on DVE, store.
#
# HW scheduling notes (engines run queues in order; emission order IS the
# schedule; every PE transpose costs ~250ns and every cross-engine sem wait
# ~0.4us, so PE work besides the matmuls must go elsewhere):
#  - ALL bf16 transposes (mask^T, K^T, Q^T) go through the XBAR DMA
#    transpose. Its layout [p, m*128+r] -> [r, m, p] puts even/odd 128-col
#    blocks in partition halves; K^T lands directly in the row-group-paired
#    layout used for concurrent K=64 matmuls via tile_position, and Q^T is
#    transposed twice (once 128-col-shifted via a padded copy) so each
#    partition half holds either q-block parity.
#  - q-passes are by q-block PARITY: pass 0 = q blocks {0,2,4,6}, pass 1 =
#    {1,3,5,7} (un-shuffled at the output store via a strided DMA AP).
#  - Per batch we emit exp(b), mult(b), qk(b+1), then pv(b) so the next
#    score matmul is never queued behind pv(b) (which waits on the DVE
#    mult). Normalize/store of unit u is deferred into unit u+1.

import os
from contextlib import ExitStack

import numpy as np

import concourse.bacc as bacc
import concourse.bass as bass
import concourse.mybir as mybir
import concourse.tile as tile
from concourse import bass2jax
from concourse.masks import make_identity

F32 = mybir.dt.float32
BF = mybir.dt.bfloat16
AF = mybir.ActivationFunctionType

N_CORES = 8
H = 8        # heads per core
SQ = 1024    # q rows per core
SK = 4096    # kv rows
D = 64       # head dim
EB = 3       # t-blocks per exp batch (3 PSUM banks)


def build_bass(H=H, SQ=SQ, SK=SK, D=D, EB=EB, sc_bufs=2, attn_bufs=3,
               out_dma="sync", prep_dma_eng="sync", repeat=1,
               emit="pipe", stage="full") -> bass.Bass:
    TB = SK // 128    # t-blocks
    QB = SQ // 128    # q-blocks of 128
    QW = min(512, SQ)  # q-pass width
    QP = SQ // QW     # q-passes (by q-block parity)
    QC = QW // 128    # 128-chunks per q-pass
    SCALE = D ** -0.5
    HTB = TB // 2     # paired t-block slots
    assert QP in (1, 2)
    nc = bacc.Bacc("TRN2")
    q_d = nc.dram_tensor("queries", [H, SQ, D], F32, kind="ExternalInput")
    k_d = nc.dram_tensor("keys", [H, SK, D], F32, kind="ExternalInput")
    v_d = nc.dram_tensor("values", [H, SK, D], F32, kind="ExternalInput")
    m_d = nc.dram_tensor("mask", [SQ, SK], F32, kind="ExternalInput")
    o_d = nc.dram_tensor("out", [H, SQ, D], F32, kind="ExternalOutput")

    with tile.TileContext(nc) as tc, ExitStack() as ctx:
        singles = ctx.enter_context(tc.tile_pool(name="singles", bufs=1))

        id_f32 = singles.tile([128, 128], F32)
        make_identity(nc, id_f32)

        # Resident exp(mask^T), contiguous per (qp, tb) so the mult operand
        # stays flat: ET[p, qp, tb, c*128+q'] = exp(mask[(2c+qp)*128+q',
        # tb*128 + p])   (q-block-parity pass layout)
        ET = singles.tile([128, QP, TB, QW], BF)

        sbg = None
        if stage == "expsbuf":
            sbg = singles.tile([128, EB, 512], F32)
            nc.gpsimd.memset(sbg, 0.25)

        psc = ctx.enter_context(tc.tile_pool(name="psc", bufs=sc_bufs, space="PSUM"))
        ppv = ctx.enter_context(tc.tile_pool(name="ppv", bufs=1, space="PSUM"))
        kpool = ctx.enter_context(tc.tile_pool(name="kpool", bufs=2))
        ktpool = ctx.enter_context(tc.tile_pool(name="ktpool", bufs=2))
        qpool = ctx.enter_context(tc.tile_pool(name="qpool", bufs=2))
        vpool = ctx.enter_context(tc.tile_pool(name="vpool", bufs=2))
        # attnp/outp opened lazily after rep-0 phase A (mpool is big)
        lazy = {}

        def hi_bf(dram_ap):
            """bf16 view of the high 2 bytes of each f32 (truncating cast,
            done by the DMA itself — no staging, no gpsimd cast)."""
            b = dram_ap.bitcast(BF)   # [..., 2*D]
            return b.rearrange("s (d two) -> s d two", two=2)[:, :, 1]

        def prep_dma(h):
            """DMA loads + gpsimd casts + K^T/Q^T xbar transposes, head h."""
            dma = getattr(nc, prep_dma_eng)
            kfp = kpool.tile([128, TB, D], F32, tag="kfp")
            dma.dma_start(out=kfp, in_=k_d[h].rearrange("(b p) d -> p b d", p=128))
            qfp = qpool.tile([128, QB, D], F32, tag="qfp")
            dma.dma_start(out=qfp, in_=q_d[h].rearrange("(b p) d -> p b d", p=128))
            vfp = vpool.tile([128, TB, D], F32, tag="vfp")
            dma.dma_start(out=vfp, in_=v_d[h].rearrange("(b p) d -> p b d", p=128))
            if stage == "loads":
                return None
            kbf = kpool.tile([128, TB, D], BF, tag="kbf")
            nc.gpsimd.tensor_copy(out=kbf, in_=kfp)
            # qbf padded one block on each side for the shifted transpose
            qbf = qpool.tile([128, QB + 2, D], BF, tag="qbf")
            nc.gpsimd.memset(qbf[:, 0, :], 0.0)
            nc.gpsimd.memset(qbf[:, QB + 1, :], 0.0)
            nc.gpsimd.tensor_copy(out=qbf[:, 1:QB + 1, :], in_=qfp)
            v1 = vpool.tile([128, TB, D + 1], BF, tag="v1")
            nc.gpsimd.tensor_copy(out=v1[:, :, 0:D], in_=vfp)
            nc.gpsimd.memset(v1[:, :, D:D + 1], 1.0)
            # kt[0:64, s, :] = K^T of t-block 2s; kt[64:128, s, :] = 2s+1
            kt = ktpool.tile([128, HTB, 128], BF, tag="kt")
            dma.dma_start_transpose(
                kt, kbf.rearrange("p a b -> p (a b)"))
            # qtA from qbf[1:QB+1]: half0 = even q-blocks, half1 = odd
            # qtB from qbf[0:QB+2]: half0 = {pad,1,3,..}, half1 = {0,2,..}
            qtA = qpool.tile([128, QB // 2, 128], BF, tag="qtA")
            dma.dma_start_transpose(
                qtA, qbf[:, 1:QB + 1, :].rearrange("p a b -> p (a b)"))
            qtB = qpool.tile([128, QB // 2 + 1, 128], BF, tag="qtB")
            dma.dma_start_transpose(
                qtB, qbf.rearrange("p a b -> p (a b)"))
            return (kt, qtA, qtB, v1)

        NBATCH = (TB + EB - 1) // EB

        def emit_qk(ktqv, qp, ib):
            kt, qtA, qtB, _ = ktqv
            tbs = list(range(ib * EB, min((ib + 1) * EB, TB)))
            sc = psc.tile([128, EB, QW], F32, tag="sc")
            if stage != "exponly":
                for j, tb in enumerate(tbs):
                    hf = (tb % 2) * 64
                    # moving: q-blocks of parity qp, at partitions hf:hf+64
                    if (tb % 2) == qp:
                        mv = qtA[hf:hf + 64, 0:QC, :]
                    elif qp == 0:
                        mv = qtB[hf:hf + 64, 0:QC, :]
                    else:
                        mv = qtB[hf:hf + 64, 1:QC + 1, :]
                    nc.tensor.matmul(sc[:, j, :], kt[hf:hf + 64, tb // 2, :], mv)
            return sc, tbs

        def make_norm(h, qp, pv):
            def norm():
                # pv is out^T [65, QW] (row 64 = denom)
                pvs = outp.tile([D + 1, QW], F32, tag="pvs")
                nc.vector.tensor_copy(out=pvs, in_=pv)
                ot_ps = ppv.tile([128, QC, D + 1], F32, tag="ot")
                for cq in range(QC):
                    nc.tensor.transpose(
                        ot_ps[:, cq, :],
                        pvs[:, cq * 128:(cq + 1) * 128],
                        id_f32[0:D + 1, 0:D + 1],
                    )
                rden = outp.tile([128, QC], F32, tag="rden")
                nc.vector.reciprocal(out=rden, in_=ot_ps[:, :, D])
                osb = outp.tile([128, QC, D], F32, tag="osb")
                for cq in range(QC):
                    nc.vector.tensor_scalar_mul(
                        osb[:, cq, :], ot_ps[:, cq, 0:D], rden[:, cq:cq + 1]
                    )
                # un-shuffle parity passes: rows of o_d = c*256 + two*128 + p
                getattr(nc, out_dma).dma_start(
                    out=o_d[h].rearrange(
                        "(c two p) d -> two p c d", two=2, p=128)[qp],
                    in_=osb,
                )
            return norm

        for _rep in range(repeat):
            # ---------------- Phase A: build E^T ----------------
            pend0 = {}
            with tc.tile_pool(name="mpool", bufs=2 if _rep == 0 else 1) as mpool:
                for qb in range(QB):
                    if qb == 1:
                        pend0["ktqv"] = prep_dma(0)
                    mbf = mpool.tile([128, SK], BF, tag="mbf")
                    HS = SK // 2
                    for mh in range(2):
                        mfp = mpool.tile([128, HS], F32, tag="mfp")
                        nc.sync.dma_start(
                            out=mfp,
                            in_=m_d[qb * 128:(qb + 1) * 128,
                                    mh * HS:(mh + 1) * HS],
                        )
                        if stage == "loads":
                            continue
                        nc.gpsimd.tensor_copy(
                            out=mbf[:, mh * HS:(mh + 1) * HS], in_=mfp)
                    if stage == "loads":
                        continue
                    # xbar transpose -> [t', tb, q] then exp -> ET
                    tmt = mpool.tile([128, TB, 128], BF, tag="tmt")
                    nc.sync.dma_start_transpose(tmt, mbf)
                    nc.scalar.activation(
                        out=ET[:, qb % 2, :, (qb // 2) * 128:(qb // 2 + 1) * 128],
                        in_=tmt,
                        func=AF.Exp,
                    )
            if "attnp" not in lazy:
                lazy["attnp"] = ctx.enter_context(
                    tc.tile_pool(name="attnp", bufs=attn_bufs))
                lazy["outp"] = ctx.enter_context(
                    tc.tile_pool(name="outp", bufs=2))
            attnp, outp = lazy["attnp"], lazy["outp"]
            if stage == "loads":
                if _rep == 0:
                    osb0 = outp.tile([128, QC, D], F32, tag="osb")
                    nc.gpsimd.memset(osb0, 0.0)
                    getattr(nc, out_dma).dma_start(
                        out=o_d[0][0:QW, :].rearrange("(c p) d -> p c d", p=128),
                        in_=osb0,
                    )
                continue

            if stage == "prep":
                for h in range(1, H):
                    prep_dma(h)
                continue

            # ------------- Phase B: flat pipelined stream over units -------
            units = [(h, qp) for h in range(H) for qp in range(QP)]
            ktqv = {0: pend0["ktqv"]}
            norm_pend = []
            nxt = {}   # pre-emitted scores for the upcoming batch/unit

            for ui, (h, qp) in enumerate(units):
                if qp == 0 and h + 1 < H:
                    ktqv[h + 1] = prep_dma(h + 1)
                v1 = ktqv[h][3]
                pv = ppv.tile([D + 1, QW], F32, tag="pv")
                if emit == "pipe" and "sc" not in nxt:
                    nxt["sc"], nxt["tbs"] = emit_qk(ktqv[h], qp, 0)
                for ib in range(NBATCH):
                    if emit == "natural":
                        sc, tbs = emit_qk(ktqv[h], qp, ib)
                    else:
                        sc, tbs = nxt["sc"], nxt["tbs"]
                    nb = len(tbs)
                    attn = attnp.tile([128, EB, QW], BF, tag="attn")
                    if stage != "qktonly":
                        nc.scalar.activation(
                            out=attn[:, :nb, :],
                            in_=sbg[:, :nb, :] if stage == "expsbuf"
                            else sc[:, :nb, :],
                            func=AF.Exp,
                            scale=SCALE,
                        )
                    if stage in ("noemult", "exponly", "qktonly"):
                        attnm = attn
                    else:
                        attnm = attnp.tile([128, EB, QW], BF, tag="attnm")
                        nc.vector.tensor_mul(
                            attnm[:, :nb, :],
                            attn[:, :nb, :],
                            ET[:, qp, tbs[0]:tbs[0] + nb, :],
                        )
                    # pre-emit next batch's score matmuls (maybe next unit's)
                    if emit == "pipe":
                        if ib + 1 < NBATCH:
                            nxt["sc"], nxt["tbs"] = emit_qk(ktqv[h], qp, ib + 1)
                        elif ui + 1 < len(units):
                            nh, nqp = units[ui + 1]
                            nxt["sc"], nxt["tbs"] = emit_qk(ktqv[nh], nqp, 0)
                        else:
                            nxt = {}
                    if ib == 0 and norm_pend:
                        norm_pend.pop(0)()
                    if stage not in ("nopv", "exponly", "qktonly"):
                        for j, tb in enumerate(tbs):
                            nc.tensor.matmul(
                                pv,
                                v1[:, tb, :],
                                attnm[:, j, :],
                                start=(tb == 0),
                                stop=(tb == TB - 1),
                                skip_group_check=True,
                            )
                if stage in ("nopv", "exponly", "qktonly", "nonorm"):
                    continue
                norm_pend.append(make_norm(h, qp, pv))
                if ui + 1 == len(units):
                    norm_pend.pop(0)()
    nc.compile()
    return nc


_NC_CACHE = None


def _get_nc():
    global _NC_CACHE
    if _NC_CACHE is None:
        _NC_CACHE = build_bass()
    return _NC_CACHE


def kernel(queries, keys, values, mask):
    queries = np.ascontiguousarray(np.asarray(queries), dtype=np.float32)
    keys = np.ascontiguousarray(np.asarray(keys), dtype=np.float32)
    values = np.ascontiguousarray(np.asarray(values), dtype=np.float32)
    mask = np.ascontiguousarray(np.asarray(mask), dtype=np.float32)

    nc = _get_nc()
    in_maps = []
    for c in range(N_CORES):
        hg, qg = divmod(c, 4)
        in_maps.append(
            {
                "queries": np.ascontiguousarray(
                    queries[hg * H:(hg + 1) * H, qg * SQ:(qg + 1) * SQ, :]
                ),
                "keys": np.ascontiguousarray(keys[hg * H:(hg + 1) * H]),
                "values": np.ascontiguousarray(values[hg * H:(hg + 1) * H]),
                "mask": np.ascontiguousarray(mask[qg * SQ:(qg + 1) * SQ, :]),
            }
        )
    results = bass2jax.run_bass_via_pjrt(nc, in_maps, n_cores=N_CORES)
    out = np.zeros((2 * H, 4 * SQ, D), np.float32)
    for c in range(N_CORES):
        hg, qg = divmod(c, 4)
        out[hg * H:(hg + 1) * H, qg * SQ:(qg + 1) * SQ, :] = results[c]["out"]
    return out


# revision 29
# speedup vs baseline: 1.0386x; 1.0386x over previous
# SDPA (naive, additive mask) for TRN2, 8 NeuronCores.
#
# Full problem: q/k/v [16, 4096, 64] f32, mask [4096, 4096] f32
#   out = softmax(q @ k^T / 8 + mask) @ v
#
# Sharding (2 head-groups x 4 q-groups = 8 cores, minimizes HBM traffic):
#   core c: hg, qg = divmod(c, 4)
#   heads hg*8:(hg+1)*8, q-rows qg*1024:(qg+1)*1024, k/v full, mask q-slice.
#
# Kernel (per core, flash-style with transposed scores):
#   E^T = exp(mask^T) resident in SBUF (bf16)  [t-major so softmax sum is a
#   matmul axis]; per head: scores^T = K^T.T @ Q^T on PE (bf16), exp on ACT
#   (scale=0.125 folded in), multiply by E^T on DVE (all-bf16 packed mode),
#   then PV = [V | ones].T @ attn^T accumulated in PSUM -> unnormalized out^T
#   plus softmax denominators in the last row; transpose back on PE, scale by
#   reciprocal on DVE, store.
#
# HW scheduling notes (engines run queues in order; emission order IS the
# schedule; every PE transpose costs ~250ns and every cross-engine sem wait
# ~0.4us, so PE work besides the matmuls must go elsewhere):
#  - ALL bf16 transposes (mask^T, K^T, Q^T) go through the XBAR DMA
#    transpose. Its layout [p, m*128+r] -> [r, m, p] puts even/odd 128-col
#    blocks in partition halves; K^T lands directly in the row-group-paired
#    layout used for concurrent K=64 matmuls via tile_position, and Q^T is
#    transposed twice (once 128-col-shifted via a padded copy) so each
#    partition half holds either q-block parity.
#  - q-passes are by q-block PARITY: pass 0 = q blocks {0,2,4,6}, pass 1 =
#    {1,3,5,7} (un-shuffled at the output store via a strided DMA AP).
#  - Per batch we emit exp(b), mult(b), qk(b+1), then pv(b) so the next
#    score matmul is never queued behind pv(b) (which waits on the DVE
#    mult). Normalize/store of unit u is deferred into unit u+1.

import os
from contextlib import ExitStack

import numpy as np

import concourse.bacc as bacc
import concourse.bass as bass
import concourse.mybir as mybir
import concourse.tile as tile
from concourse import bass2jax
from concourse.masks import make_identity

F32 = mybir.dt.float32
BF = mybir.dt.bfloat16
AF = mybir.ActivationFunctionType

N_CORES = 8
H = 8        # heads per core
SQ = 1024    # q rows per core
SK = 4096    # kv rows
D = 64       # head dim
EB = 3       # t-blocks per exp batch (3 PSUM banks)


def build_bass(H=H, SQ=SQ, SK=SK, D=D, EB=EB, sc_bufs=2, attn_bufs=3,
               out_dma="sync", prep_dma_eng="sync", repeat=1,
               emit="pipe", stage="full") -> bass.Bass:
    TB = SK // 128    # t-blocks
    QB = SQ // 128    # q-blocks of 128
    QW = min(512, SQ)  # q-pass width
    QP = SQ // QW     # q-passes (by q-block parity)
    QC = QW // 128    # 128-chunks per q-pass
    SCALE = D ** -0.5
    HTB = TB // 2     # paired t-block slots
    assert QP in (1, 2)
    nc = bacc.Bacc("TRN2")
    q_d = nc.dram_tensor("queries", [H, SQ, D], F32, kind="ExternalInput")
    k_d = nc.dram_tensor("keys", [H, SK, D], F32, kind="ExternalInput")
    v_d = nc.dram_tensor("values", [H, SK, D], F32, kind="ExternalInput")
    m_d = nc.dram_tensor("mask", [SQ, SK], F32, kind="ExternalInput")
    o_d = nc.dram_tensor("out", [H, SQ, D], F32, kind="ExternalOutput")

    with tile.TileContext(nc) as tc, ExitStack() as ctx:
        singles = ctx.enter_context(tc.tile_pool(name="singles", bufs=1))

        id_f32 = singles.tile([128, 128], F32)
        make_identity(nc, id_f32)

        # Resident exp(mask^T), contiguous per (qp, tb) so the mult operand
        # stays flat: ET[p, qp, tb, c*128+q'] = exp(mask[(2c+qp)*128+q',
        # tb*128 + p])   (q-block-parity pass layout)
        ET = singles.tile([128, QP, TB, QW], BF)

        sbg = None
        if stage == "expsbuf":
            sbg = singles.tile([128, EB, 512], F32)
            nc.gpsimd.memset(sbg, 0.25)

        psc = ctx.enter_context(tc.tile_pool(name="psc", bufs=sc_bufs, space="PSUM"))
        ppv = ctx.enter_context(tc.tile_pool(name="ppv", bufs=1, space="PSUM"))
        kpool = ctx.enter_context(tc.tile_pool(name="kpool", bufs=2))
        ktpool = ctx.enter_context(tc.tile_pool(name="ktpool", bufs=2))
        qpool = ctx.enter_context(tc.tile_pool(name="qpool", bufs=2))
        vpool = ctx.enter_context(tc.tile_pool(name="vpool", bufs=2))
        # attnp/outp opened lazily after rep-0 phase A (mpool is big)
        lazy = {}

        def hi_bf(dram_ap):
            """bf16 view of the high 2 bytes of each f32 (truncating cast,
            done by the DMA itself — no staging, no gpsimd cast)."""
            b = dram_ap.bitcast(BF)   # [..., 2*D]
            return b.rearrange("s (d two) -> s d two", two=2)[:, :, 1]

        def prep_dma(h):
            """DMA loads + gpsimd casts + K^T/Q^T xbar transposes, head h."""
            dma = getattr(nc, prep_dma_eng)
            kfp = kpool.tile([128, TB, D], F32, tag="kfp")
            dma.dma_start(out=kfp, in_=k_d[h].rearrange("(b p) d -> p b d", p=128))
            qfp = qpool.tile([128, QB, D], F32, tag="qfp")
            dma.dma_start(out=qfp, in_=q_d[h].rearrange("(b p) d -> p b d", p=128))
            vfp = vpool.tile([128, TB, D], F32, tag="vfp")
            dma.dma_start(out=vfp, in_=v_d[h].rearrange("(b p) d -> p b d", p=128))
            if stage == "loads":
                return None
            kbf = kpool.tile([128, TB, D], BF, tag="kbf")
            nc.gpsimd.tensor_copy(out=kbf, in_=kfp)
            # qbf padded one block on each side for the shifted transpose
            qbf = qpool.tile([128, QB + 2, D], BF, tag="qbf")
            nc.gpsimd.memset(qbf[:, 0, :], 0.0)
            nc.gpsimd.memset(qbf[:, QB + 1, :], 0.0)
            nc.gpsimd.tensor_copy(out=qbf[:, 1:QB + 1, :], in_=qfp)
            v1 = vpool.tile([128, TB, D + 1], BF, tag="v1")
            nc.gpsimd.tensor_copy(out=v1[:, :, 0:D], in_=vfp)
            nc.gpsimd.memset(v1[:, :, D:D + 1], 1.0)
            # kt[0:64, s, :] = K^T of t-block 2s; kt[64:128, s, :] = 2s+1
            # (xbar transposes are HWDGE-only, keep them on sync)
            kt = ktpool.tile([128, HTB, 128], BF, tag="kt")
            nc.sync.dma_start_transpose(
                kt, kbf.rearrange("p a b -> p (a b)"))
            # qtA from qbf[1:QB+1]: half0 = even q-blocks, half1 = odd
            # qtB from qbf[0:QB+2]: half0 = {pad,1,3,..}, half1 = {0,2,..}
            qtA = qpool.tile([128, QB // 2, 128], BF, tag="qtA")
            nc.sync.dma_start_transpose(
                qtA, qbf[:, 1:QB + 1, :].rearrange("p a b -> p (a b)"))
            qtB = qpool.tile([128, QB // 2 + 1, 128], BF, tag="qtB")
            nc.sync.dma_start_transpose(
                qtB, qbf.rearrange("p a b -> p (a b)"))
            return (kt, qtA, qtB, v1)

        NBATCH = (TB + EB - 1) // EB

        def emit_qk(ktqv, qp, ib):
            kt, qtA, qtB, _ = ktqv
            tbs = list(range(ib * EB, min((ib + 1) * EB, TB)))
            sc = psc.tile([128, EB, QW], F32, tag="sc")
            if stage != "exponly":
                for j, tb in enumerate(tbs):
                    hf = (tb % 2) * 64
                    # moving: q-blocks of parity qp, at partitions hf:hf+64
                    if (tb % 2) == qp:
                        mv = qtA[hf:hf + 64, 0:QC, :]
                    elif qp == 0:
                        mv = qtB[hf:hf + 64, 0:QC, :]
                    else:
                        mv = qtB[hf:hf + 64, 1:QC + 1, :]
                    nc.tensor.matmul(sc[:, j, :], kt[hf:hf + 64, tb // 2, :], mv)
            return sc, tbs

        def make_norm(h, qp, pv):
            def norm():
                # pv is out^T [65, QW] (row 64 = denom)
                pvs = outp.tile([D + 1, QW], F32, tag="pvs")
                nc.vector.tensor_copy(out=pvs, in_=pv)
                ot_ps = ppv.tile([128, QC, D + 1], F32, tag="ot")
                for cq in range(QC):
                    nc.tensor.transpose(
                        ot_ps[:, cq, :],
                        pvs[:, cq * 128:(cq + 1) * 128],
                        id_f32[0:D + 1, 0:D + 1],
                    )
                rden = outp.tile([128, QC], F32, tag="rden")
                nc.vector.reciprocal(out=rden, in_=ot_ps[:, :, D])
                osb = outp.tile([128, QC, D], F32, tag="osb")
                for cq in range(QC):
                    nc.vector.tensor_scalar_mul(
                        osb[:, cq, :], ot_ps[:, cq, 0:D], rden[:, cq:cq + 1]
                    )
                # un-shuffle parity passes: rows of o_d = c*256 + two*128 + p
                getattr(nc, out_dma).dma_start(
                    out=o_d[h].rearrange(
                        "(c two p) d -> two p c d", two=2, p=128)[qp],
                    in_=osb,
                )
            return norm

        for _rep in range(repeat):
            # ---------------- Phase A: build E^T ----------------
            pend0 = {}
            with tc.tile_pool(name="mpool", bufs=2 if _rep == 0 else 1) as mpool:
                for qb in range(QB):
                    if qb == 1:
                        pend0["ktqv"] = prep_dma(0)
                    mbf = mpool.tile([128, SK], BF, tag="mbf")
                    HS = SK // 2
                    for mh in range(2):
                        mfp = mpool.tile([128, HS], F32, tag="mfp")
                        nc.sync.dma_start(
                            out=mfp,
                            in_=m_d[qb * 128:(qb + 1) * 128,
                                    mh * HS:(mh + 1) * HS],
                        )
                        if stage == "loads":
                            continue
                        nc.gpsimd.tensor_copy(
                            out=mbf[:, mh * HS:(mh + 1) * HS], in_=mfp)
                    if stage == "loads":
                        continue
                    # xbar transpose -> [t', tb, q] then exp -> ET
                    tmt = mpool.tile([128, TB, 128], BF, tag="tmt")
                    nc.sync.dma_start_transpose(tmt, mbf)
                    nc.scalar.activation(
                        out=ET[:, qb % 2, :, (qb // 2) * 128:(qb // 2 + 1) * 128],
                        in_=tmt,
                        func=AF.Exp,
                    )
            if "attnp" not in lazy:
                lazy["attnp"] = ctx.enter_context(
                    tc.tile_pool(name="attnp", bufs=attn_bufs))
                lazy["outp"] = ctx.enter_context(
                    tc.tile_pool(name="outp", bufs=2))
            attnp, outp = lazy["attnp"], lazy["outp"]
            if stage == "loads":
                if _rep == 0:
                    osb0 = outp.tile([128, QC, D], F32, tag="osb")
                    nc.gpsimd.memset(osb0, 0.0)
                    getattr(nc, out_dma).dma_start(
                        out=o_d[0][0:QW, :].rearrange("(c p) d -> p c d", p=128),
                        in_=osb0,
                    )
                continue

            if stage == "prep":
                for h in range(1, H):
                    prep_dma(h)
                continue

            # ------------- Phase B: flat pipelined stream over units -------
            units = [(h, qp) for h in range(H) for qp in range(QP)]
            ktqv = {0: pend0["ktqv"]}
            norm_pend = []
            nxt = {}   # pre-emitted scores for the upcoming batch/unit

            for ui, (h, qp) in enumerate(units):
                if qp == 0 and h + 1 < H:
                    ktqv[h + 1] = prep_dma(h + 1)
                v1 = ktqv[h][3]
                pv = ppv.tile([D + 1, QW], F32, tag="pv")
                if emit == "pipe" and "sc" not in nxt:
                    nxt["sc"], nxt["tbs"] = emit_qk(ktqv[h], qp, 0)
                for ib in range(NBATCH):
                    if emit == "natural":
                        sc, tbs = emit_qk(ktqv[h], qp, ib)
                    else:
                        sc, tbs = nxt["sc"], nxt["tbs"]
                    nb = len(tbs)
                    attn = attnp.tile([128, EB, QW], BF, tag="attn")
                    if stage != "qktonly":
                        nc.scalar.activation(
                            out=attn[:, :nb, :],
                            in_=sbg[:, :nb, :] if stage == "expsbuf"
                            else sc[:, :nb, :],
                            func=AF.Exp,
                            scale=SCALE,
                        )
                    if stage in ("noemult", "exponly", "qktonly"):
                        attnm = attn
                    else:
                        attnm = attnp.tile([128, EB, QW], BF, tag="attnm")
                        nc.vector.tensor_mul(
                            attnm[:, :nb, :],
                            attn[:, :nb, :],
                            ET[:, qp, tbs[0]:tbs[0] + nb, :],
                        )
                    # pre-emit next batch's score matmuls (maybe next unit's)
                    if emit == "pipe":
                        if ib + 1 < NBATCH:
                            nxt["sc"], nxt["tbs"] = emit_qk(ktqv[h], qp, ib + 1)
                        elif ui + 1 < len(units):
                            nh, nqp = units[ui + 1]
                            nxt["sc"], nxt["tbs"] = emit_qk(ktqv[nh], nqp, 0)
                        else:
                            nxt = {}
                    if ib == 0 and norm_pend:
                        norm_pend.pop(0)()
                    if stage not in ("nopv", "exponly", "qktonly"):
                        for j, tb in enumerate(tbs):
                            nc.tensor.matmul(
                                pv,
                                v1[:, tb, :],
                                attnm[:, j, :],
                                start=(tb == 0),
                                stop=(tb == TB - 1),
                                skip_group_check=True,
                            )
                if stage in ("nopv", "exponly", "qktonly", "nonorm"):
                    continue
                norm_pend.append(make_norm(h, qp, pv))
                if ui + 1 == len(units):
                    norm_pend.pop(0)()
    nc.compile()
    return nc


_NC_CACHE = None


def _get_nc():
    global _NC_CACHE
    if _NC_CACHE is None:
        _NC_CACHE = build_bass()
    return _NC_CACHE


def kernel(queries, keys, values, mask):
    queries = np.ascontiguousarray(np.asarray(queries), dtype=np.float32)
    keys = np.ascontiguousarray(np.asarray(keys), dtype=np.float32)
    values = np.ascontiguousarray(np.asarray(values), dtype=np.float32)
    mask = np.ascontiguousarray(np.asarray(mask), dtype=np.float32)

    nc = _get_nc()
    in_maps = []
    for c in range(N_CORES):
        hg, qg = divmod(c, 4)
        in_maps.append(
            {
                "queries": np.ascontiguousarray(
                    queries[hg * H:(hg + 1) * H, qg * SQ:(qg + 1) * SQ, :]
                ),
                "keys": np.ascontiguousarray(keys[hg * H:(hg + 1) * H]),
                "values": np.ascontiguousarray(values[hg * H:(hg + 1) * H]),
                "mask": np.ascontiguousarray(mask[qg * SQ:(qg + 1) * SQ, :]),
            }
        )
    results = bass2jax.run_bass_via_pjrt(nc, in_maps, n_cores=N_CORES)
    out = np.zeros((2 * H, 4 * SQ, D), np.float32)
    for c in range(N_CORES):
        hg, qg = divmod(c, 4)
        out[hg * H:(hg + 1) * H, qg * SQ:(qg + 1) * SQ, :] = results[c]["out"]
    return out
